# revision 16
# baseline (speedup 1.0000x reference)
"""Trainium2 Bass kernel for nn_CSHead2 (moe_routing).

Network (per sample): two-pathway conv head over [512, 56, 56] features.
  object path : t1 = relu(conv3x3(feat*emb_obj, td_w1)); t2 = relu(conv3x3(t1, td_w2))
                obj_pred = w_obj_head @ t2 + b_obj  (1x1)
  bottom-up   : xs = conv3x3(obj_pred, w_bu_start)
                x2 = relu(conv3x3([xs; relu(conv3x3(t1,w_lat1))], w_bu1))
                bu = relu(conv3x3([x2; relu(conv3x3(t2,w_lat2))], w_bu2))
  part path   : y = relu(conv3x3(relu(conv3x3(feat*emb_i + bu, td_w1)), td_w2))
                part_pred = w_part_heads[instr] @ y + b_sel  (1x1)

Sharding: pure data parallelism — one sample per NeuronCore (B=8, 8 cores),
weights replicated, per-sample routing (embedding/head gathers) resolved on
host. No collectives.

Kernel layout: channels on partitions (4 blocks of 128), spatial plane
zero-padded to 58x58 and flattened into the free dimension. A 3x3 conv is
36 accumulated PE matmuls (4 ci-blocks x 9 taps) per (output-block, span):
shifting the input by a tap offset is just an AP offset in the flat padded
plane. Matmuls run as float32r (full PE rate at free-dim >= 256, ~1e-4
relative accuracy). Conv epilogues (bias+relu) run on the ACT engine
straight out of PSUM; head epilogues and the part-path gating run on DVE;
padding re-zeroing runs on GpSimd.
"""

import numpy as np

import bass_rust
import concourse.bass as bass
import concourse.mybir as mybir
import concourse.tile as tile

# ---- problem geometry (hardcoded per contract) ----
B, C, H, W = 8, 512, 56, 56
N_OBJ, N_OWP, P = 21, 16, 8
CB = C // 128                      # channel blocks
# Shared-pad layout: rows have stride W+1=57; the single pad column at the
# end of each row serves as BOTH the right pad of row r and the left pad of
# row r+1. One extra leading zero element covers the top-left corner tap.
WP = W + 1                         # row stride (57)
ROWS = H + 2                       # 58 rows incl. top/bottom zero rows
BASE = 1                           # leading zero element
SP = BASE + ROWS * WP + 1          # 3308: +1 so the span total stays even
INT0 = BASE + WP                   # first interior flat index (58)
# One past the last interior index, extended by one PAD position (row 56's
# shared pad col) so the total span length is even — fp32r matmuls require
# an even moving dim. The extra position is re-zeroed by the pad memsets.
INT1 = BASE + (H + 1) * WP         # 3250

F32 = mybir.dt.float32
F32R = mybir.dt.float32r
RELU = mybir.ActivationFunctionType.Relu

# free-dim spans covering the interior; all >= 256 so fp32r streams at
# 1 cycle/row, and <= 496 so each span fits one PSUM bank with headroom
SPANS = []
_s = INT0
while _s < INT1:
    _n = min(496, INT1 - _s)
    if INT1 - _s - _n and INT1 - _s - _n < 256:
        _n = INT1 - _s - 256          # keep the tail span >= 256
    SPANS.append((_s, _n))
    _s += _n

# tap offsets in the flat padded plane
TAPS = [(ky - 1) * WP + (kx - 1) for ky in range(3) for kx in range(3)]

MAX_WAITS = 1


def _split_sync_waits(nc, max_waits=MAX_WAITS):
    """This container's walrus accepts at most one sync-wait command per
    instruction; hoist excess waits onto InstNoOp carriers inserted before
    the offending instruction (same engine, so program order is kept)."""
    ctr = 0
    for f in nc.m.functions:
        for bb in f.blocks:
            insts = bb.instructions  # live list
            i = 0
            while i < len(insts):
                ins = insts[i]
                si = ins.sync_info
                if si is not None and len(si.on_wait) > max_waits:
                    waits = list(si.on_wait)
                    si.on_wait = waits[-max_waits:]
                    rest = waits[:-max_waits]
                    carriers = []
                    for j in range(0, len(rest), max_waits):
                        ctr += 1
                        nop = mybir.InstNoOp(
                            name=f"antwaitnop_{ctr}", ins=[], outs=[])
                        nop.engine = ins.engine
                        nop.sync_info = bass_rust.SyncInfo(
                            on_wait=rest[j:j + max_waits], on_update=[])
                        carriers.append(nop)
                    insts[i:i] = carriers
                    i += len(carriers)
                i += 1
    return ctr


def _rows(ap):
    """[*, SP] buffer -> [*, ROWS, WP] row view (skipping the BASE element)."""
    return ap[:, BASE:BASE + ROWS * WP].rearrange("p (h w) -> p h w", w=WP)


def _interior(ap):
    """[*, SP] buffer -> [*, H, W] interior view."""
    return _rows(ap)[:, 1:H + 1, 0:W]


def _pad_memsets(nc, block_ap):
    """Zero the padding of one [*, SP] channel-block plane: the leading
    element + top row, the bottom row, and the per-row shared pad column."""
    nc.gpsimd.memset(block_ap[:, 0:BASE + WP].bitcast(F32), 0.0)
    nc.gpsimd.memset(block_ap[:, BASE + (H + 1) * WP:SP].bitcast(F32), 0.0)
    side = block_ap[:, BASE + WP + W: BASE + WP + W + (H * WP)].rearrange(
        "p (h w) -> p h w", w=WP)[:, :, 0:1]
    nc.gpsimd.memset(side.bitcast(F32), 0.0)


def _emit_conv(nc, wpool, pspool, in_aps, wdram, cin, cout,
               out_view, bias_col=None, vecs=None, relu=True, post_block=None):
    """3x3 SAME conv: in_aps = list of [*, SP] channel-block planes (padding
    already zero), wdram = [cin, 9, cout] DRAM weights, out_view(o_blk) ->
    [128, SP] destination plane."""
    cinb = len(in_aps)
    kpart = in_aps[0].shape[0]  # 128, or 21 for the obj_pred input
    for ob in range(cout // 128):
        wts = []
        for ci in range(cinb):
            wt = wpool.tile([kpart, 9, 128], F32R, name=f"wt_{ci}", tag="w")
            nc.sync.dma_start(
                wt[:],
                wdram[ci * 128: ci * 128 + kpart, :, ob * 128:(ob + 1) * 128])
            wts.append(wt)
        ov = out_view(ob)
        for (s, n) in SPANS:
            pt = pspool.tile([128, n], F32, name="ps", tag="ps")
            last = (cinb - 1, 8)
            for ci in range(cinb):
                for k in range(9):
                    rhs = in_aps[ci][:, s + TAPS[k]: s + TAPS[k] + n]
                    nc.tensor.matmul(
                        pt[:], wts[ci][:, k, :],
                        rhs,
                        start=(ci == 0 and k == 0), stop=((ci, k) == last))
            if relu:
                nc.scalar.activation(ov[:, s:s + n], pt[:], RELU,
                                     bias=vecs[:, bias_col + ob: bias_col + ob + 1])
            else:
                nc.scalar.copy(ov[:, s:s + n], pt[:])
        _pad_memsets(nc, ov)
        if post_block is not None:
            post_block(ob, ov)


def _emit_head(nc, pspool, in4, whead, nout, out_ap, bias_ap):
    """1x1 conv head: psum[o, span] = sum_blk whead[:, blk, :].T @ in[blk]."""
    for (s, n) in SPANS:
        pt = pspool.tile([nout, n], F32, name="psh", tag="ps")
        for blk in range(CB):
            nc.tensor.matmul(
                pt[:], whead[:, blk, :],
                in4[:, blk, s:s + n],
                start=(blk == 0), stop=(blk == CB - 1))
        nc.vector.tensor_scalar_add(out_ap[:, s:s + n], pt[:], bias_ap)


def build_module(split_waits=True, reps=1):
    nc = bass.Bass("TRN2", target_bir_lowering=False, debug=False)

    dt = lambda name, shape, d=F32R: nc.dram_tensor(name, shape, d, kind="ExternalInput").ap()
    feat = dt("feat", [C, H, W])
    wg1 = dt("wg1", [C, 9, C])        # td_w1 with emb_obj folded into ci
    w1 = dt("w1", [C, 9, C])
    w2 = dt("w2", [C, 9, C])
    wlat1 = dt("wlat1", [C, 9, C // 2])
    wlat2 = dt("wlat2", [C, 9, C // 2])
    wbu1 = dt("wbu1", [C, 9, C // 2])
    wbu2 = dt("wbu2", [C, 9, C])
    wbus = dt("wbus", [N_OBJ, 9, C // 2])
    wobj = dt("wobj", [C, N_OBJ])
    wsel = dt("wsel", [C, P])
    vecs_d = dt("vecs", [128, 24], F32)
    obj_out = nc.dram_tensor("obj_out", [N_OBJ, H, W], F32, kind="ExternalOutput").ap()
    part_out = nc.dram_tensor("part_out", [P, H, W], F32, kind="ExternalOutput").ap()

    with tile.TileContext(nc) as tc:
        import contextlib
        loop_ctx = tc.For_i(0, reps, 1) if reps > 1 else contextlib.nullcontext()
        with loop_ctx, \
             tc.tile_pool(name="acts", bufs=3) as acts, \
             tc.tile_pool(name="hd", bufs=1) as hdp, \
             tc.tile_pool(name="wp", bufs=8) as wpool, \
             tc.tile_pool(name="sm", bufs=1) as sm, \
             tc.tile_pool(name="fr", bufs=1) as frp, \
             tc.tile_pool(name="ps", bufs=8, space="PSUM") as ps:

            # ---- small constants ----
            vecs = sm.tile([128, 24], F32, name="vecs", tag="vecs")
            nc.sync.dma_start(vecs[:], vecs_d[:])
            whead_obj = sm.tile([128, CB, N_OBJ], F32R, name="whead_obj", tag="who")
            whead_sel = sm.tile([128, CB, P], F32R, name="whead_sel", tag="whs")
            for blk in range(CB):
                nc.sync.dma_start(whead_obj[:, blk, :], wobj[blk * 128:(blk + 1) * 128, :])
                nc.sync.dma_start(whead_sel[:, blk, :], wsel[blk * 128:(blk + 1) * 128, :])

            # ---- S0: load features into padded planes ----
            ft = acts.tile([128, CB, SP], F32R, name="feat_t", tag="a4")
            for blk in range(CB):
                _pad_memsets(nc, ft[:, blk])
                for r0 in (0, H // 2):
                    nc.sync.dma_start(
                        _interior(ft[:, blk])[:, r0:r0 + H // 2],
                        feat[blk * 128:(blk + 1) * 128, r0:r0 + H // 2])

            def blocks(t):
                return [t[:, b] for b in range(CB)]

            # ---- S1: t1 = relu(conv(feat, wg1) + td_b1) ----
            t1 = acts.tile([128, CB, SP], F32R, name="t1", tag="a4")
            _emit_conv(nc, wpool, ps, blocks(ft), wg1, C, C,
                       lambda ob: t1[:, ob], bias_col=0, vecs=vecs)

            # ---- S2: t2 = relu(conv(t1, w2) + td_b2) ----
            t2 = acts.tile([128, CB, SP], F32R, name="t2", tag="a4")
            _emit_conv(nc, wpool, ps, blocks(t1), w2, C, C,
                       lambda ob: t2[:, ob], bias_col=4, vecs=vecs)

            # ---- S3: lat1 = relu(conv(t1, wlat1) + b_lat1) -> bu1in[2:4] ----
            bu1in = acts.tile([128, CB, SP], F32R, name="bu1in", tag="a4")
            _emit_conv(nc, wpool, ps, blocks(t1), wlat1, C, C // 2,
                       lambda ob: bu1in[:, 2 + ob], bias_col=8, vecs=vecs)

            # ---- S4: obj head + output ----
            objp = hdp.tile([N_OBJ, SP], F32R, name="objp", tag="hd")
            _emit_head(nc, ps, t2, whead_obj, N_OBJ, objp, vecs[:N_OBJ, 18:19])
            _pad_memsets(nc, objp)
            nc.sync.dma_start(obj_out[:], _interior(objp).bitcast(F32))

            # ---- S5: lat2 = relu(conv(t2, wlat2) + b_lat2) -> bu2in[2:4] ----
            bu2in = acts.tile([128, CB, SP], F32R, name="bu2in", tag="a4")
            _emit_conv(nc, wpool, ps, blocks(t2), wlat2, C, C // 2,
                       lambda ob: bu2in[:, 2 + ob], bias_col=10, vecs=vecs)

            # ---- S6: bu_start conv (21 -> 256, no bias/relu) -> bu1in[0:2] ----
            _emit_conv(nc, wpool, ps, [objp], wbus, N_OBJ, C // 2,
                       lambda ob: bu1in[:, ob], relu=False)

            # ---- S7: x2 = relu(conv(bu1in, wbu1) + b_bu1) -> bu2in[0:2] ----
            _emit_conv(nc, wpool, ps, blocks(bu1in), wbu1, C, C // 2,
                       lambda ob: bu2in[:, ob], bias_col=12, vecs=vecs)

            # ---- S8 + S9: bu = relu(conv(bu2in, wbu2) + b_bu2), then gate
            # each block (y0 = feat*emb_i + bu, in place) as soon as it is
            # produced so the DVE work overlaps the remaining conv ----
            NB = 4  # row bands per block
            RB = H // NB

            def gate_block(blk, ov):
                for b in range(NB):
                    fr = frp.tile([128, RB * W], F32R, name="fr", tag="fr")
                    nc.sync.dma_start(
                        fr.rearrange("p (h w) -> p h w", w=W),
                        feat[blk * 128:(blk + 1) * 128, b * RB:(b + 1) * RB, :])
                    nc.vector.tensor_scalar_mul(fr[:], fr[:], vecs[:, 20 + blk:21 + blk])
                    bv = _rows(ov)[:, 1 + b * RB:1 + (b + 1) * RB, 0:W]
                    nc.vector.tensor_add(
                        bv, bv, fr.rearrange("p (h w) -> p h w", w=W))

            bu = acts.tile([128, CB, SP], F32R, name="bu", tag="a4")
            _emit_conv(nc, wpool, ps, blocks(bu2in), wbu2, C, C,
                       lambda ob: bu[:, ob], bias_col=14, vecs=vecs,
                       post_block=gate_block)

            # ---- S10/S11: part-path td convs (shared weights w1, w2) ----
            y1 = acts.tile([128, CB, SP], F32R, name="y1", tag="a4")
            _emit_conv(nc, wpool, ps, blocks(bu), w1, C, C,
                       lambda ob: y1[:, ob], bias_col=0, vecs=vecs)
            y2 = acts.tile([128, CB, SP], F32R, name="y2", tag="a4")
            _emit_conv(nc, wpool, ps, blocks(y1), w2, C, C,
                       lambda ob: y2[:, ob], bias_col=4, vecs=vecs)

            # ---- S12: part head + output ----
            partp = hdp.tile([P, SP], F32, name="partp", tag="hd")
            _emit_head(nc, ps, y2, whead_sel, P, partp, vecs[:P, 19:20])
            nc.sync.dma_start(part_out[:], _interior(partp))

    if split_waits:
        _split_sync_waits(nc)
    return nc


def _prep_host(inputs):
    """Transpose/fold weights and resolve per-sample routing on host."""
    f32 = np.float32

    def conv_t(w):  # [O, I, 3, 3] -> [I, 9, O] contiguous
        return np.ascontiguousarray(
            np.transpose(np.asarray(w, f32), (1, 2, 3, 0)).reshape(
                w.shape[1], 9, w.shape[0]))

    emb = np.asarray(inputs["emb"], f32)
    emb_obj = emb[N_OWP]
    td_w1 = np.asarray(inputs["td_w1"], f32)
    shared = {
        "wg1": conv_t(td_w1 * emb_obj[None, :, None, None]),
        "w1": conv_t(td_w1),
        "w2": conv_t(inputs["td_w2"]),
        "wlat1": conv_t(inputs["w_lat1"]),
        "wlat2": conv_t(inputs["w_lat2"]),
        "wbu1": conv_t(inputs["w_bu1"]),
        "wbu2": conv_t(inputs["w_bu2"]),
        "wbus": conv_t(inputs["w_bu_start"]),
        "wobj": np.ascontiguousarray(np.asarray(inputs["w_obj_head"], f32).T),
    }

    instr = np.asarray(inputs["instruction"]).astype(np.int64)
    feats = np.asarray(inputs["features"], f32)
    w_ph = np.asarray(inputs["w_part_heads"], f32)
    b_ph = np.asarray(inputs["b_part_heads"], f32)

    in_maps = []
    for b in range(B):
        ib = int(instr[b])
        vecs = np.zeros((128, 24), f32)
        vecs[:, 0:4] = np.asarray(inputs["td_b1"], f32).reshape(4, 128).T
        vecs[:, 4:8] = np.asarray(inputs["td_b2"], f32).reshape(4, 128).T
        vecs[:, 8:10] = np.asarray(inputs["b_lat1"], f32).reshape(2, 128).T
        vecs[:, 10:12] = np.asarray(inputs["b_lat2"], f32).reshape(2, 128).T
        vecs[:, 12:14] = np.asarray(inputs["b_bu1"], f32).reshape(2, 128).T
        vecs[:, 14:18] = np.asarray(inputs["b_bu2"], f32).reshape(4, 128).T
        vecs[:N_OBJ, 18] = np.asarray(inputs["b_obj_head"], f32)
        vecs[:P, 19] = b_ph[ib]
        vecs[:, 20:24] = emb[ib].reshape(4, 128).T
        in_maps.append({
            "feat": np.ascontiguousarray(feats[b]),
            "wsel": np.ascontiguousarray(w_ph[ib].T),
            "vecs": vecs,
            **shared,
        })
    return in_maps


# per-core inputs are sharded over the core mesh axis; everything else
# (the big weight arrays) is passed replicated, avoiding the 8x host-side
# concatenation a plain run_bass_kernel_spmd would do.
PER_CORE = ("feat", "wsel", "vecs")

_CACHE = {}


def _build_runner():
    import jax
    from jax.sharding import Mesh, PartitionSpec
    from jax.experimental.shard_map import shard_map
    from concourse import bass2jax

    nc = build_module()
    bass2jax.install_neuronx_cc_hook()
    in_names, out_names, out_avals = [], [], []
    for alloc in nc.m.functions[0].allocations:
        if not isinstance(alloc, mybir.MemoryLocationSet):
            continue
        name = alloc.memorylocations[0].name
        if alloc.kind == "ExternalInput":
            if nc.partition_id_tensor is None or name != nc.partition_id_tensor.name:
                in_names.append(name)
        elif alloc.kind == "ExternalOutput":
            out_names.append(name)
            out_avals.append(jax.core.ShapedArray(
                tuple(alloc.tensor_shape), mybir.dt.np(alloc.dtype)))
    all_names = list(in_names) + out_names
    pname = nc.partition_id_tensor.name if nc.partition_id_tensor else None
    if pname is not None:
        all_names.append(pname)

    def _body(*args):
        operands = list(args)
        if pname is not None:
            operands.append(bass2jax.partition_id_tensor())
        return tuple(bass2jax._bass_exec_p.bind(
            *operands, out_avals=tuple(out_avals), in_names=tuple(all_names),
            out_names=tuple(out_names), lowering_input_output_aliases=(),
            sim_require_finite=True, sim_require_nnan=True, nc=nc))

    devices = jax.devices()[:B]
    mesh = Mesh(np.asarray(devices), ("core",))
    shard, repl = PartitionSpec("core"), PartitionSpec()
    in_specs = tuple(shard if nm in PER_CORE else repl for nm in in_names) \
        + (shard,) * len(out_names)
    fn = jax.jit(shard_map(_body, mesh=mesh, in_specs=in_specs,
                           out_specs=(shard,) * len(out_names), check_rep=False),
                 keep_unused=True)
    return fn, in_names, out_names, out_avals


def kernel(**inputs):
    if "runner" not in _CACHE:
        _CACHE["runner"] = _build_runner()
    fn, in_names, out_names, out_avals = _CACHE["runner"]
    in_maps = _prep_host(inputs)
    args = []
    for nm in in_names:
        if nm in PER_CORE:
            args.append(np.concatenate([m[nm] for m in in_maps], axis=0))
        else:
            args.append(in_maps[0][nm])
    for av in out_avals:
        args.append(np.zeros((B * av.shape[0], *av.shape[1:]), av.dtype))
    outs = fn(*args)
    res = {nm: np.asarray(o).reshape(B, *out_avals[i].shape)
           for i, (nm, o) in enumerate(zip(out_names, outs))}
    return res["obj_out"].astype(np.float32), res["part_out"].astype(np.float32)
